# revision 3
# baseline (speedup 1.0000x reference)
"""Trainium2 Bass kernel for nn_DilatedOCA (dilated overlapping cross-attention).

Math (per reference):
  xn = x / sqrt(var(x, ch) + 1e-5) * ln_w           (bias-free LN over channels)
  qkv = w_qkv @ xn (1x1 conv); q/k/v split
  q: [heads, N=4096, 16] from channels
  k,v: torch-unfold(12x12 win, stride 8, pad 2) + a reshape that scrambles
       (channel, window-pos, window-idx) into [heads, M=9216, 16] where the
       "16" dim is the LOW 4 BITS OF THE WINDOW INDEX (faithful to source).
  attn = softmax(q k^T / 4) over all M; out = attn @ v; final 1x1 conv w_out.

Index algebra (head h, d = 8a+s with a=d//8, s=d%8):
  K^T[d, m] = k_pad[16h+ci, 16lq+8a+kh, 8s+kw]   m=(ci,kh,kw,lq)
  V[m, d]   = v_pad[16h+ci, 16lq+8a+kh, 8s+kw]
  Q^T[d, n] = q[16h+d, n]
Softmax/PV are invariant to any fixed permutation of m, so we use our own
enumeration  m' = ((((khH*3+khL)*4+lq)*6+kw1)*2+kw0)*16+ci  (kh=3khH+khL,
kw=2kw1+kw0), which makes the K/V gather DMAs contiguous 192-element runs.

Sharding: 8 cores = 4 heads x 2 query-halves (n in [0,2048) / [2048,4096)).
Per core: LN (stats in transposed orientation; rstd folded into the conv
output scaling), qkv conv, K/V gather, flash-style attention in bf16 (no
max-subtraction: logits ~N(0,0.4), exp cannot overflow), per-head final-conv
partial y_h = w_out[:, head] @ out_h^T.  Host sums 4 head partials per half.
"""

import sys

for _p in ("/opt/trn_rl_repo", "/root/.axon_site/_ro/pypackages"):
    if _p not in sys.path:
        sys.path.insert(0, _p)

import numpy as np

import concourse.bass as bass
import concourse.mybir as mybir
import concourse.tile as tile
from concourse import bacc
from concourse.bass_utils import run_bass_kernel_spmd

F32 = mybir.dt.float32
BF16 = mybir.dt.float16  # fp16: same PE rate as bf16, 8x mantissa
AF = mybir.ActivationFunctionType
ALU = mybir.AluOpType

HEADS, DH = 4, 16
NPIX, NHALF = 4096, 2048
PADW = 68          # padded image height/width
PFREE = PADW * 16  # padT3 free size: col*16 + ci = 1088
M = 9216           # keys per head
NT = 72            # m' tiles of 128
EPS = 1e-5

_CACHE = {}


def _build(stage="full", reps=1):
    nc = bacc.Bacc(trn_type="TRN2")
    dbg_d = None
    if stage != "full":
        dbg_d = nc.dram_tensor("dbg", [128, M], F32, kind="ExternalOutput")

    x_d = nc.dram_tensor("x", [64, NPIX], F32, kind="ExternalInput")
    xq_d = nc.dram_tensor("xq", [64, NHALF], F32, kind="ExternalInput")
    wkvT_d = nc.dram_tensor("wkvT", [64, 32], F32, kind="ExternalInput")
    wqT_d = nc.dram_tensor("wqT", [64, 16], F32, kind="ExternalInput")
    woutT_d = nc.dram_tensor("woutT", [16, 64], F32, kind="ExternalInput")
    ones1_d = nc.dram_tensor("ones1", [1, 64], F32, kind="ExternalInput")
    id128_d = nc.dram_tensor("id128", [128, 128], F32, kind="ExternalInput")
    id17_d = nc.dram_tensor("id17", [17, 17], F32, kind="ExternalInput")
    onesM_d = nc.dram_tensor("onesM", [1, M], F32, kind="ExternalInput")
    y_d = nc.dram_tensor("y", [64, NHALF], F32, kind="ExternalOutput")
    ktmp_d = nc.dram_tensor("ktmp", [NPIX, 16], F32)
    vtmp_d = nc.dram_tensor("vtmp", [NPIX, 16], F32)

    with tile.TileContext(nc) as tc:
        with tc.tile_pool(name="sb", bufs=1) as sb:
            # persistent sbuf tensors
            xsb = sb.tile([64, NPIX], F32)
            xqsb = sb.tile([64, NHALF], F32)
            padk = sb.tile([PADW, PFREE], F32)
            padv = sb.tile([PADW, PFREE], F32)
            gk = sb.tile([16, M], F32)
            gkb = sb.tile([16, M], BF16)
            gv = sb.tile([17, M], F32)
            vt_all = sb.tile([128, 17 * NT], BF16)
            qsb = sb.tile([16, NHALF], BF16)
            stgkv = sb.tile([128, 1024], F32)
            stats = sb.tile([128, 96], F32)   # s1 cols 0:48, s2 cols 48:96
            rstdT = sb.tile([128, 48], F32)   # col t: chunk t (32 x, 16 xq)
            osb = sb.tile([128, 512], F32)
            ysb = sb.tile([64, NHALF], F32)
            wkvT = sb.tile([64, 32], F32)
            wqT = sb.tile([64, 16], F32)
            woutT = sb.tile([16, 64], F32)
            ones1 = sb.tile([1, 64], F32)
            id128 = sb.tile([128, 128], F32)
            id17 = sb.tile([17, 17], F32)

            for dst, src in (
                (xsb, x_d), (xqsb, xq_d), (wkvT, wkvT_d), (wqT, wqT_d),
                (woutT, woutT_d), (ones1, ones1_d), (id128, id128_d),
                (id17, id17_d),
            ):
                nc.sync.dma_start(out=dst[:, :], in_=src[:, :])

            # border zeros for padded images; ones row for the softmax denom
            nc.gpsimd.memset(padk[:, :], 0.0)
            nc.gpsimd.memset(padv[:, :], 0.0)
            nc.sync.dma_start(out=gv[16:17, :], in_=onesM_d[:, :])

            with tc.tile_pool(name="sm", bufs=3) as sm, \
                 tc.tile_pool(name="pre", bufs=3, space="PSUM") as pre:

                # ---- LN stats in transposed (pixel-partition) orientation --
                def chunk_src(t):
                    if t < 32:
                        return xsb[:, 128 * t:128 * (t + 1)]
                    return xqsb[:, 128 * (t - 32):128 * (t - 31)]

                for t in range(48):
                    trp = pre.tile([128, 64], F32, tag="pre")
                    nc.tensor.transpose(trp[:, :], chunk_src(t),
                                        id128[0:64, 0:64])
                    xT = sm.tile([128, 64], F32, tag="xT")
                    nc.vector.tensor_copy(xT[:, :], trp[:, :])
                    nc.vector.reduce_sum(stats[:, t:t + 1], xT[:, :],
                                         axis=mybir.AxisListType.X)
                    scr = sm.tile([128, 64], F32, tag="scr")
                    nc.vector.tensor_mul(scr[:, :], xT[:, :], xT[:, :])
                    nc.vector.reduce_sum(stats[:, 48 + t:49 + t], scr[:, :],
                                         axis=mybir.AxisListType.X)

                # rstd = 1/sqrt(s2/64 - (s1/64)^2 + eps)   [128, 48]
                mean = sm.tile([128, 48], F32, tag="mean")
                nc.vector.tensor_scalar_mul(mean[:, :], stats[:, 0:48], 1.0 / 64)
                nc.vector.tensor_mul(mean[:, :], mean[:, :], mean[:, :])
                varr = sm.tile([128, 48], F32, tag="varr")
                nc.vector.tensor_scalar_mul(varr[:, :], stats[:, 48:96], 1.0 / 64)
                nc.vector.tensor_sub(varr[:, :], varr[:, :], mean[:, :])
                nc.vector.tensor_scalar_add(varr[:, :], varr[:, :], EPS)
                nc.scalar.activation(rstdT[:, :], varr[:, :], AF.Sqrt)
                nc.vector.reciprocal(rstdT[:, :], rstdT[:, :])

                # ---- k,v 1x1 conv on RAW x; rstd folded into psum scaling --
                for t in range(32):
                    kv = pre.tile([128, 32], F32, tag="pre")
                    nc.tensor.matmul(kv[:, :], xsb[:, 128 * t:128 * (t + 1)],
                                     wkvT[:, :], start=True, stop=True)
                    nc.vector.tensor_scalar_mul(
                        stgkv[:, 32 * t:32 * (t + 1)], kv[:, :],
                        rstdT[:, t:t + 1])

                # stgkv[p, 32t + c0 + ci] = (k|v)[ci, pixel=128t+p]
                # -> (k|v)tmp[pixel, ci]  (DRAM, pixel-major)
                for tmp_d, c0 in ((ktmp_d, 0), (vtmp_d, 16)):
                    src_ap = bass.AP(tensor=stgkv.tensor, offset=c0,
                                     ap=[[1024, 128], [32, 32], [1, 16]])
                    dst_ap = bass.AP(tensor=tmp_d, offset=0,
                                     ap=[[16, 128], [2048, 32], [1, 16]])
                    nc.sync.dma_start(out=dst_ap, in_=src_ap)
                # -> pad[row, (col+2)*16 + ci] interior (rows/cols +2 offset)
                for tmp_d, pad_t in ((ktmp_d, padk), (vtmp_d, padv)):
                    src_ap = bass.AP(tensor=tmp_d, offset=0,
                                     ap=[[1024, 64], [1, 1024]])
                    dst_ap = bass.AP(tensor=pad_t.tensor,
                                     offset=2 * PFREE + 2 * 16,
                                     ap=[[PFREE, 64], [1, 1024]])
                    nc.sync.dma_start(out=dst_ap, in_=src_ap)

                if stage == "pads":
                    nc.sync.dma_start(out=bass.AP(tensor=dbg_d, offset=0,
                                                  ap=[[M, PADW], [1, PFREE]]),
                                      in_=padk[:, :])
                    nc.sync.dma_start(
                        out=bass.AP(tensor=dbg_d, offset=2048,
                                    ap=[[M, PADW], [1, PFREE]]),
                        in_=padv[:, :])

                # ---- q conv (head slice, 0.25 prefolded), pixel-part -------
                for t in range(16):
                    qp = pre.tile([128, 16], F32, tag="pre")
                    nc.tensor.matmul(qp[:, :], xqsb[:, 128 * t:128 * (t + 1)],
                                     wqT[:, :], start=True, stop=True)
                    qTc = sm.tile([128, 16], F32, tag="qTc")
                    nc.vector.tensor_scalar_mul(qTc[:, :], qp[:, :],
                                                rstdT[:, 32 + t:33 + t])
                    qp2 = pre.tile([16, 128], F32, tag="pre")
                    nc.tensor.transpose(qp2[:, :], qTc[:, :], id128[:, :])
                    nc.vector.tensor_copy(qsb[:, 128 * t:128 * (t + 1)],
                                          qp2[:, :])

                # ---- gathers: pad -> G  (48 DMAs each) ---------------------
                # G[8a+s, m'] = pad[16lq+3khH+khL+8a, (8s+2kw1+kw0)*16+ci]
                for pad_t, g_t in ((padk, gk), (padv, gv)):
                    for khH in range(4):
                        for khL in range(3):
                            for lq in range(4):
                                row0 = 16 * lq + 3 * khH + khL
                                src_ap = bass.AP(
                                    tensor=pad_t.tensor, offset=row0 * PFREE,
                                    ap=[[8 * PFREE, 2], [128, 8], [1, 192]])
                                dst_ap = bass.AP(
                                    tensor=g_t.tensor,
                                    offset=2304 * khH + 768 * khL + 192 * lq,
                                    ap=[[M, 16], [1, 192]])
                                nc.sync.dma_start(out=dst_ap, in_=src_ap)

                if stage == "gather":
                    nc.sync.dma_start(out=dbg_d[0:16, :], in_=gk[:, :])
                    nc.sync.dma_start(out=dbg_d[32:49, :], in_=gv[:, :])

                # bf16 cast of K^T for full-rate PE matmuls
                nc.vector.tensor_copy(gkb[:, :], gk[:, :])

                # ---- V^T tiles: [17, 128] -> [128, 17] via PE transpose ----
                for T in range(NT):
                    vtp = pre.tile([128, 17], F32, tag="pre")
                    nc.tensor.transpose(vtp[:, :], gv[:, 128 * T:128 * (T + 1)],
                                        id17[:, :])
                    nc.vector.tensor_copy(vt_all[:, 17 * T:17 * (T + 1)],
                                          vtp[:, :])

            if stage == "vt":
                nc.gpsimd.dma_start(out=dbg_d[:, 0:17 * NT], in_=vt_all[:, :])
            if stage in ("pads", "gather", "vt"):
                nc.vector.memset(ysb[:, :], 0.0)
                nc.sync.dma_start(out=y_d[:, :], in_=ysb[:, :])
            else:
                _run_main(nc, tc, stage, dbg_d, sb, gkb, gv, vt_all, qsb, osb,
                          ysb, woutT, ones1, y_d, reps)

    nc.compile()
    return nc


def _run_main(nc, tc, stage, dbg_d, sb, gkb, gv, vt_all, qsb, osb,
              ysb, woutT, ones1, y_d, reps=1):
    with tc.tile_pool(name="spool", bufs=2, space="PSUM") as spool, \
         tc.tile_pool(name="pvp", bufs=1, space="PSUM") as pvp, \
         tc.tile_pool(name="pp", bufs=3) as pp:
        pv = pvp.tile([128, 512], F32)
        for _rep in range(reps):
          for t3 in range(24):
            for nck in range(4):
                ncs = slice(512 * nck, 512 * (nck + 1))
                st = spool.tile([128, 1536], F32, tag="s")
                for g in range(3):
                    T = 3 * t3 + g
                    nc.tensor.matmul(
                        st[:, 512 * g:512 * (g + 1)],
                        gkb[:, 128 * T:128 * (T + 1)],
                        qsb[:, ncs], start=True, stop=True)
                pt = pp.tile([128, 1536], BF16, tag="p")
                nc.scalar.activation(pt[:, :], st[:, :], AF.Exp)
                for g in range(3):
                    T = 3 * t3 + g
                    nc.tensor.matmul(
                        pv[32 * nck:32 * nck + 17, :],
                        vt_all[:, 17 * T:17 * (T + 1)],
                        pt[:, 512 * g:512 * (g + 1)],
                        start=(t3 == 0 and g == 0),
                        stop=(t3 == 23 and g == 2),
                        tile_position=(0, 32 * nck))

        # ---- normalize + final 1x1 conv -------------------------------
        # matmul operands must sit at base partition 0: DMA each
        # accumulator strip [17, 512] down from partitions 32*nck.
        nc.vector.tensor_copy(osb[:, :], pv[:, :])
        if stage == "pv":
            nc.sync.dma_start(out=dbg_d[:, 0:512], in_=osb[:, :])
        num = sb.tile([16, 512], F32)
        den = sb.tile([1, 512], F32)
        bsb = sb.tile([64, 512], F32)
        for nck in range(4):
            src_num = bass.AP(tensor=osb.tensor, offset=32 * nck * 512,
                              ap=[[512, 16], [1, 512]])
            src_den = bass.AP(tensor=osb.tensor,
                              offset=(32 * nck + 16) * 512,
                              ap=[[512, 1], [1, 512]])
            nc.sync.dma_start(out=num[:, :], in_=src_num)
            nc.sync.dma_start(out=den[:, :], in_=src_den)
            nc.vector.reciprocal(den[:, :], den[:, :])
            yp = spool.tile([64, 512], F32, tag="s")
            nc.tensor.matmul(yp[:, :], woutT[:, :], num[:, :],
                             start=True, stop=True)
            bp = spool.tile([64, 512], F32, tag="s")
            nc.tensor.matmul(bp[:, :], ones1[:, :], den[:, :],
                             start=True, stop=True)
            # DVE has one PSUM read port: stage bp in SBUF first
            nc.vector.tensor_copy(bsb[:, :], bp[:, :])
            nc.vector.tensor_mul(ysb[:, 512 * nck:512 * (nck + 1)],
                                 yp[:, :], bsb[:, :])
        nc.sync.dma_start(out=y_d[:, :], in_=ysb[:, :])


def _get_nc():
    if "nc" not in _CACHE:
        _CACHE["nc"] = _build()
    return _CACHE["nc"]


def kernel(x, w_qkv, w_out, ln_w, _want_trace=False, _tmpdir=None):
    x = np.asarray(x, np.float32)
    w_qkv = np.asarray(w_qkv, np.float32)
    w_out = np.asarray(w_out, np.float32)
    ln_w = np.asarray(ln_w, np.float32)

    x2d = np.ascontiguousarray(x.reshape(64, NPIX))
    ones1 = np.ones((1, 64), np.float32)
    id128 = np.eye(128, dtype=np.float32)
    id17 = np.eye(17, dtype=np.float32)
    onesM = np.ones((1, M), np.float32)

    in_maps = []
    for c in range(8):
        h, half = c % 4, c // 4
        wq = w_qkv[16 * h:16 * h + 16, :]
        wk = w_qkv[64 + 16 * h:64 + 16 * h + 16, :]
        wv = w_qkv[128 + 16 * h:128 + 16 * h + 16, :]
        lw = ln_w[None, :]
        in_maps.append({
            "x": x2d,
            "xq": np.ascontiguousarray(x2d[:, NHALF * half:NHALF * (half + 1)]),
            "wkvT": np.ascontiguousarray(
                (np.concatenate([wk, wv], 0) * lw).T.astype(np.float32)),
            "wqT": np.ascontiguousarray((0.25 * wq * lw).T.astype(np.float32)),
            "woutT": np.ascontiguousarray(
                w_out[:, 16 * h:16 * h + 16].T.astype(np.float32)),
            "ones1": ones1,
            "id128": id128,
            "id17": id17,
            "onesM": onesM,
        })

    nc = _get_nc()
    res = run_bass_kernel_spmd(nc, in_maps, list(range(8)), trace=_want_trace,
                               tmpdir=_tmpdir)
    if _want_trace:
        _CACHE["last_result"] = res

    y = np.empty((64, NPIX), np.float32)
    for half in range(2):
        acc = np.zeros((64, NHALF), np.float32)
        for h in range(4):
            acc += res.results[4 * half + h]["y"]
        y[:, NHALF * half:NHALF * (half + 1)] = acc
    return y.reshape(1, 64, 64, 64)



# revision 13
# speedup vs baseline: 1.4100x; 1.4100x over previous
"""Trainium2 Bass kernel for nn_DilatedOCA (dilated overlapping cross-attention).

Math (per reference):
  xn = x / sqrt(var(x, ch) + 1e-5) * ln_w           (bias-free LN over channels)
  qkv = w_qkv @ xn (1x1 conv); q/k/v split
  q: [heads, N=4096, 16] from channels
  k,v: torch-unfold(12x12 win, stride 8, pad 2) + a reshape that scrambles
       (channel, window-pos, window-idx) into [heads, M=9216, 16] where the
       "16" dim is the LOW 4 BITS OF THE WINDOW INDEX (faithful to source).
  attn = softmax(q k^T / 4) over all M; out = attn @ v; final 1x1 conv w_out.

Index algebra (head h, d = 8a+s with a=d//8, s=d%8):
  K^T[d, m] = k_pad[16h+ci, 16lq+8a+kh, 8s+kw]   m=(ci,kh,kw,lq)
  V[m, d]   = v_pad[16h+ci, 16lq+8a+kh, 8s+kw]
  Q^T[d, n] = q[16h+d, n]
Softmax/PV are invariant to any fixed permutation of m, so we use our own
enumeration  m' = ((((khH*3+khL)*4+lq)*6+kw1)*2+kw0)*16+ci  (kh=3khH+khL,
kw=2kw1+kw0), which makes the K/V gather DMAs contiguous 192-element runs.

Sharding: 8 cores = 4 heads x 2 query-halves (n in [0,2048) / [2048,4096)).

Perf structure (v2):
  - LN stats via N=1 matmuls (s1/s2 columns), rstd = Exp(-0.5*Ln(var+eps))
    -> single ACT table set (natural_log_exp) for the whole kernel.
  - k/v images staged fp16; gathers lq-merged (24 DMAs) and spread across
    sync/vector/gpsimd queues; K^T replicated at partition bases 0/32/64/96.
  - Main loop ACT-bound: per group of 3 key-tiles, 3 row-packed S matmuls
    (tile_position=(32r,0), K=16 contraction) -> one exp [128,1536] -> 3 PV
    matmuls col-packed at (0,32*nck).  Emission is software-pipelined so the
    scalar engine's exp stream never waits on the PE.
  - V^T tile transposes for later chunks interleave into main-loop PE slack.
  - 1/den via Exp(-Ln(den)) on ACT (DVE reciprocal is slow; ACT rsqrt banned).
"""

import sys

for _p in ("/opt/trn_rl_repo", "/root/.axon_site/_ro/pypackages"):
    if _p not in sys.path:
        sys.path.insert(0, _p)

import numpy as np

import concourse.bass as bass
import concourse.mybir as mybir
import concourse.tile as tile
from concourse import bacc
from concourse.bass_utils import run_bass_kernel_spmd

F32 = mybir.dt.float32
F16 = mybir.dt.float16  # fp16: same PE rate as bf16, 8x mantissa
AF = mybir.ActivationFunctionType

HEADS, DH = 4, 16
NPIX, NHALF = 4096, 2048
PADW = 68          # padded image height/width
PFREE = PADW * 16  # pad free size: col*16 + ci = 1088
M = 9216           # keys per head
NT = 72            # m' tiles of 128
NCHUNK = 4         # khH chunks
TCH = NT // NCHUNK     # 18 key-tiles per chunk
MCH = M // NCHUNK      # 2304 m' per chunk
EPS = 1e-5
ROWPACK = True

_CACHE = {}


def _build(reps=1):
    nc = bacc.Bacc(trn_type="TRN2")

    x_d = nc.dram_tensor("x", [64, NPIX], F32, kind="ExternalInput")
    xq_d = nc.dram_tensor("xq", [64, NHALF], F32, kind="ExternalInput")
    wkvT_d = nc.dram_tensor("wkvT", [64, 32], F32, kind="ExternalInput")
    wqT_d = nc.dram_tensor("wqT", [64, 16], F32, kind="ExternalInput")
    woutT_d = nc.dram_tensor("woutT", [16, 64], F32, kind="ExternalInput")
    ones1_d = nc.dram_tensor("ones1", [1, 64], F32, kind="ExternalInput")
    ones64_d = nc.dram_tensor("ones64", [64, 1], F32, kind="ExternalInput")
    id128_d = nc.dram_tensor("id128", [128, 128], F32, kind="ExternalInput")
    id17_d = nc.dram_tensor("id17", [17, 17], F16, kind="ExternalInput")
    onesM_d = nc.dram_tensor("onesM", [1, M], F16, kind="ExternalInput")
    y_d = nc.dram_tensor("y", [64, NHALF], F32, kind="ExternalOutput")
    ktmp_d = nc.dram_tensor("ktmp", [NPIX, 16], F16)
    vtmp_d = nc.dram_tensor("vtmp", [NPIX, 16], F16)

    with tile.TileContext(nc) as tc:
        with tc.tile_pool(name="sb", bufs=1) as sb:
            # persistent sbuf tensors
            xsb = sb.tile([64, NPIX], F32)
            xqsb = sb.tile([64, NHALF], F32)
            xx = sb.tile([64, NPIX], F32)
            xxq = sb.tile([64, NHALF], F32)
            padk = sb.tile([PADW, PFREE], F16)
            padv = sb.tile([PADW, PFREE], F16)
            gkb4 = [sb.tile([128, MCH], F16, name=f"gkb4_{c}")
                    for c in range(NCHUNK)]
            gvc = [sb.tile([17, MCH], F16, name=f"gvc_{c}")
                   for c in range(NCHUNK)]
            vtc = [sb.tile([128, 17 * TCH], F16, name=f"vtc_{c}")
                   for c in range(NCHUNK)]
            qsb4 = sb.tile([128, NHALF], F16)
            stgkv = sb.tile([128, 1024], F32)
            stg16 = sb.tile([128, 1024], F16)
            rstdT = sb.tile([128, 48], F32)   # col t: chunk t (32 x, 16 xq)
            osb = sb.tile([128, 512], F32)
            den4 = sb.tile([4, 512], F32)
            dinv = sb.tile([4, 512], F32)
            num = sb.tile([16, 512], F32)
            den1 = sb.tile([1, 512], F32)
            bsb = sb.tile([64, 512], F32)
            ysb = sb.tile([64, NHALF], F32)
            wkvT = sb.tile([64, 32], F32)
            wqT = sb.tile([64, 16], F32)
            woutT = sb.tile([16, 64], F32)
            ones1 = sb.tile([1, 64], F32)
            ones64 = sb.tile([64, 1], F32)
            id128 = sb.tile([128, 128], F32)
            id17 = sb.tile([17, 17], F16)

            # input DMAs spread over queues
            nc.sync.dma_start(out=xsb[:, :], in_=x_d[:, :])
            nc.sync.dma_start(out=xqsb[:, :], in_=xq_d[:, :])
            for dst, src in ((wkvT, wkvT_d), (wqT, wqT_d), (woutT, woutT_d),
                             (ones1, ones1_d), (ones64, ones64_d)):
                nc.scalar.dma_start(out=dst[:, :], in_=src[:, :])
            nc.gpsimd.dma_start(out=id128[:, :], in_=id128_d[:, :])
            nc.gpsimd.dma_start(out=id17[:, :], in_=id17_d[:, :])
            nc.gpsimd.memset(padk[:, :], 0.0)
            nc.gpsimd.memset(padv[:, :], 0.0)
            for c in range(NCHUNK):
                nc.gpsimd.dma_start(out=gvc[c][16:17, :],
                                    in_=onesM_d[:, MCH * c:MCH * (c + 1)])

            with tc.tile_pool(name="sm", bufs=4) as sm, \
                 tc.tile_pool(name="pre", bufs=2, space="PSUM") as pre:

                # ---- LN stats: s1/s2 per 128-pixel chunk via N=1 matmuls ---
                nc.vector.tensor_mul(xx[:, :], xsb[:, :], xsb[:, :])
                nc.vector.tensor_mul(xxq[:, :], xqsb[:, :], xqsb[:, :])

                def chunk_src(t, sq):
                    a, b = (xx, xxq) if sq else (xsb, xqsb)
                    if t < 32:
                        return a[:, 128 * t:128 * (t + 1)]
                    return b[:, 128 * (t - 32):128 * (t - 31)]

                s12p = pre.tile([128, 96], F32, tag="st12")
                for t in range(48):
                    nc.tensor.matmul(s12p[:, t:t + 1], chunk_src(t, False),
                                     ones64[:, :], start=True, stop=True)
                for t in range(48):
                    nc.tensor.matmul(s12p[:, 48 + t:49 + t], chunk_src(t, True),
                                     ones64[:, :], start=True, stop=True)

                # rstd = exp(-0.5*ln(s2/64 - (s1/64)^2 + eps))
                mean = sm.tile([128, 48], F32, tag="mean")
                nc.vector.tensor_scalar_mul(mean[:, :], s12p[:, 0:48], 1.0 / 64)
                nc.vector.tensor_mul(mean[:, :], mean[:, :], mean[:, :])
                varr = sm.tile([128, 48], F32, tag="varr")
                nc.vector.tensor_scalar_mul(varr[:, :], s12p[:, 48:96], 1.0 / 64)
                nc.vector.tensor_sub(varr[:, :], varr[:, :], mean[:, :])
                nc.vector.tensor_scalar_add(varr[:, :], varr[:, :], EPS)
                lnv = sm.tile([128, 48], F32, tag="lnv")
                nc.scalar.activation(lnv[:, :], varr[:, :], AF.Ln)
                nc.scalar.activation(rstdT[:, :], lnv[:, :], AF.Exp, scale=-0.5)

                # ---- k,v 1x1 conv on RAW x; rstd folded in psum scaling ----
                for t in range(32):
                    kv = pre.tile([128, 32], F32, tag="kv")
                    nc.tensor.matmul(kv[:, :], xsb[:, 128 * t:128 * (t + 1)],
                                     wkvT[:, :], start=True, stop=True)
                    nc.vector.tensor_scalar_mul(
                        stgkv[:, 32 * t:32 * (t + 1)], kv[:, :],
                        rstdT[:, t:t + 1])
                nc.vector.tensor_copy(stg16[:, :], stgkv[:, :])

                # stg16[p, 32t + c0 + ci] = (k|v)[ci, pixel=128t+p]
                # -> (k|v)tmp[pixel, ci]  (DRAM, pixel-major, fp16)
                for tmp_d, c0 in ((ktmp_d, 0), (vtmp_d, 16)):
                    src_ap = bass.AP(tensor=stg16.tensor, offset=c0,
                                     ap=[[1024, 128], [32, 32], [1, 16]])
                    dst_ap = bass.AP(tensor=tmp_d, offset=0,
                                     ap=[[16, 128], [2048, 32], [1, 16]])
                    nc.sync.dma_start(out=dst_ap, in_=src_ap)
                # -> pad[row, (col+2)*16 + ci] interior (rows/cols +2 offset)
                for tmp_d, pad_t, eng in ((ktmp_d, padk, nc.sync),
                                          (vtmp_d, padv, nc.scalar)):
                    src_ap = bass.AP(tensor=tmp_d, offset=0,
                                     ap=[[1024, 64], [1, 1024]])
                    dst_ap = bass.AP(tensor=pad_t.tensor,
                                     offset=2 * PFREE + 2 * 16,
                                     ap=[[PFREE, 64], [1, 1024]])
                    eng.dma_start(out=dst_ap, in_=src_ap)

                # ---- q conv (head slice, 0.25 prefolded), pixel-part ------
                for t in range(16):
                    qp = pre.tile([128, 16], F32, tag="qp")
                    nc.tensor.matmul(qp[:, :], xqsb[:, 128 * t:128 * (t + 1)],
                                     wqT[:, :], start=True, stop=True)
                    qTc = sm.tile([128, 16], F32, tag="qTc")
                    nc.vector.tensor_scalar_mul(qTc[:, :], qp[:, :],
                                                rstdT[:, 32 + t:33 + t])
                    qp2 = pre.tile([16, 128], F32, tag="qp2")
                    nc.tensor.transpose(qp2[:, :], qTc[:, :], id128[:, :])
                    nc.vector.tensor_copy(qsb4[0:16, 128 * t:128 * (t + 1)],
                                          qp2[:, :])
                if ROWPACK:
                    for r in (1, 2, 3):
                        nc.sync.dma_start(
                            out=bass.AP(tensor=qsb4.tensor, offset=32 * r * NHALF,
                                        ap=[[NHALF, 16], [1, NHALF]]),
                            in_=bass.AP(tensor=qsb4.tensor, offset=0,
                                        ap=[[NHALF, 16], [1, NHALF]]))

                # ---- gathers: pad -> gkb4/gv chunks (lq merged, fp16) ------
                # G[8a+s, m'] = pad[16lq+3khH+khL+8a, (8s+2kw1+kw0)*16+ci]
                for c in range(NCHUNK):
                    for khL in range(3):
                        for lq in range(4):
                            for pad_t, gdst, eng in (
                                    (padk, gkb4[c], nc.sync),
                                    (padv, gvc[c], nc.scalar)):
                                src_ap = bass.AP(
                                    tensor=pad_t.tensor,
                                    offset=(16 * lq + 3 * c + khL) * PFREE,
                                    ap=[[8 * PFREE, 2], [128, 8], [1, 192]])
                                dst_ap = bass.AP(
                                    tensor=gdst.tensor,
                                    offset=768 * khL + 192 * lq,
                                    ap=[[MCH, 16], [1, 192]])
                                eng.dma_start(out=dst_ap, in_=src_ap)
                    if ROWPACK:
                        for r in (1, 2, 3):
                            nc.gpsimd.dma_start(
                                out=bass.AP(tensor=gkb4[c].tensor,
                                            offset=32 * r * MCH,
                                            ap=[[MCH, 16], [1, MCH]]),
                                in_=bass.AP(tensor=gkb4[c].tensor, offset=0,
                                            ap=[[MCH, 16], [1, MCH]]))

            _run_main(nc, tc, sb, gkb4, gvc, vtc, qsb4, osb, den4, dinv, num,
                      den1, bsb, ysb, woutT, ones1, id17, y_d, reps)

    nc.compile()
    return nc


def _run_main(nc, tc, sb, gkb4, gvc, vtc, qsb4, osb, den4, dinv, num, den1,
              bsb, ysb, woutT, ones1, id17, y_d, reps=1):
    NG = 24  # groups of 3 key-tiles
    with tc.tile_pool(name="stp", bufs=2, space="PSUM") as stp, \
         tc.tile_pool(name="pvp", bufs=1, space="PSUM") as pvp, \
         tc.tile_pool(name="pvt", bufs=1, space="PSUM") as pvtp, \
         tc.tile_pool(name="pp", bufs=3) as pp:
        pv = pvp.tile([128, 512], F32)

        def do_vt(c, i):
            vtp = pvtp.tile([128, 17], F16, tag="vt")
            nc.tensor.transpose(vtp[:, :], gvc[c][:, 128 * i:128 * (i + 1)],
                                id17[:, :])
            nc.vector.tensor_copy(vtc[c][:, 17 * i:17 * (i + 1)], vtp[:, :])

        for i in range(TCH):
            do_vt(0, i)
        vt_tasks = [(c, i) for c in range(1, NCHUNK) for i in range(TCH)]
        vt_tasks.reverse()  # pop() from chunk 1 upward

        def emit_S(g, nck):
            st = stp.tile([128, 1536], F32, tag="s")
            for j in range(3):
                T = 3 * g + j
                c, tcix = T // TCH, T % TCH
                r = T % 4 if ROWPACK else 0
                nc.tensor.matmul(
                    st[:, 512 * j:512 * (j + 1)],
                    gkb4[c][32 * r:32 * r + 16, 128 * tcix:128 * (tcix + 1)],
                    qsb4[32 * r:32 * r + 16, 512 * nck:512 * (nck + 1)],
                    start=True, stop=True,
                    tile_position=(32 * r, 0) if ROWPACK else None)
            return st

        for _rep in range(reps):
            pend = emit_S(0, 0)
            for g in range(NG):
                for nck in range(4):
                    st = pend
                    pt = pp.tile([128, 1536], F16, tag="p")
                    nc.scalar.activation(pt[:, :], st[:, :], AF.Exp)
                    ng, nn = (g, nck + 1) if nck < 3 else (g + 1, 0)
                    if ng < NG:
                        pend = emit_S(ng, nn)
                    # fill PE idle slot (waiting on exp) with V^T transposes
                    if g < 8:
                        for _ in range(2):
                            if vt_tasks:
                                do_vt(*vt_tasks.pop())
                    for j in range(3):
                        T = 3 * g + j
                        c, tcix = T // TCH, T % TCH
                        nc.tensor.matmul(
                            pv[32 * nck:32 * nck + 17, :],
                            vtc[c][:, 17 * tcix:17 * (tcix + 1)],
                            pt[:, 512 * j:512 * (j + 1)],
                            start=(g == 0 and j == 0),
                            stop=(g == NG - 1 and j == 2),
                            tile_position=(0, 32 * nck))

        # ---- normalize + final 1x1 conv -------------------------------
        nc.vector.tensor_copy(osb[:, :], pv[:, :])
        # den strips live at partitions 32*nck+16; 1/den = exp(-ln(den))
        nc.sync.dma_start(
            out=den4[:, :],
            in_=bass.AP(tensor=osb.tensor, offset=16 * 512,
                        ap=[[32 * 512, 4], [1, 512]]))
        nc.scalar.activation(den4[:, :], den4[:, :], AF.Ln)
        nc.scalar.activation(dinv[:, :], den4[:, :], AF.Exp, scale=-1.0)
        for nck in range(4):
            nc.sync.dma_start(
                out=num[:, :],
                in_=bass.AP(tensor=osb.tensor, offset=32 * nck * 512,
                            ap=[[512, 16], [1, 512]]))
            nc.sync.dma_start(
                out=den1[:, :],
                in_=bass.AP(tensor=dinv.tensor, offset=nck * 512,
                            ap=[[512, 1], [1, 512]]))
            yp = stp.tile([64, 512], F32, tag="s")
            nc.tensor.matmul(yp[:, :], woutT[:, :], num[:, :],
                             start=True, stop=True)
            bp = stp.tile([64, 512], F32, tag="s")
            nc.tensor.matmul(bp[:, :], ones1[:, :], den1[:, :],
                             start=True, stop=True)
            # DVE has one PSUM read port: stage bp in SBUF first
            nc.vector.tensor_copy(bsb[:, :], bp[:, :])
            nc.vector.tensor_mul(ysb[:, 512 * nck:512 * (nck + 1)],
                                 yp[:, :], bsb[:, :])
        nc.sync.dma_start(out=y_d[:, :], in_=ysb[:, :])


def _get_nc():
    if "nc" not in _CACHE:
        _CACHE["nc"] = _build()
    return _CACHE["nc"]


def kernel(x, w_qkv, w_out, ln_w, _want_trace=False, _tmpdir=None):
    x = np.asarray(x, np.float32)
    w_qkv = np.asarray(w_qkv, np.float32)
    w_out = np.asarray(w_out, np.float32)
    ln_w = np.asarray(ln_w, np.float32)

    x2d = np.ascontiguousarray(x.reshape(64, NPIX))
    ones1 = np.ones((1, 64), np.float32)
    ones64 = np.ones((64, 1), np.float32)
    id128 = np.eye(128, dtype=np.float32)
    id17 = np.eye(17, dtype=np.float16)
    onesM = np.ones((1, M), np.float16)

    in_maps = []
    for c in range(8):
        h, half = c % 4, c // 4
        wq = w_qkv[16 * h:16 * h + 16, :]
        wk = w_qkv[64 + 16 * h:64 + 16 * h + 16, :]
        wv = w_qkv[128 + 16 * h:128 + 16 * h + 16, :]
        lw = ln_w[None, :]
        in_maps.append({
            "x": x2d,
            "xq": np.ascontiguousarray(x2d[:, NHALF * half:NHALF * (half + 1)]),
            "wkvT": np.ascontiguousarray(
                (np.concatenate([wk, wv], 0) * lw).T.astype(np.float32)),
            "wqT": np.ascontiguousarray((0.25 * wq * lw).T.astype(np.float32)),
            "woutT": np.ascontiguousarray(
                w_out[:, 16 * h:16 * h + 16].T.astype(np.float32)),
            "ones1": ones1,
            "ones64": ones64,
            "id128": id128,
            "id17": id17,
            "onesM": onesM,
        })

    nc = _get_nc()
    res = run_bass_kernel_spmd(nc, in_maps, list(range(8)), trace=_want_trace,
                               tmpdir=_tmpdir)
    if _want_trace:
        _CACHE["last_result"] = res

    y = np.empty((64, NPIX), np.float32)
    for half in range(2):
        acc = np.zeros((64, NHALF), np.float32)
        for h in range(4):
            acc += res.results[4 * half + h]["y"]
        y[:, NHALF * half:NHALF * (half + 1)] = acc
    return y.reshape(1, 64, 64, 64)


# revision 16
# speedup vs baseline: 1.5064x; 1.0683x over previous
"""Trainium2 Bass kernel for nn_DilatedOCA (dilated overlapping cross-attention).

Math (per reference):
  xn = x / sqrt(var(x, ch) + 1e-5) * ln_w           (bias-free LN over channels)
  qkv = w_qkv @ xn (1x1 conv); q/k/v split
  q: [heads, N=4096, 16] from channels
  k,v: torch-unfold(12x12 win, stride 8, pad 2) + a reshape that scrambles
       (channel, window-pos, window-idx) into [heads, M=9216, 16] where the
       "16" dim is the LOW 4 BITS OF THE WINDOW INDEX (faithful to source).
  attn = softmax(q k^T / 4) over all M; out = attn @ v; final 1x1 conv w_out.

Index algebra (head h, d = 8a+s with a=d//8, s=d%8):
  K^T[d, m] = k_pad[16h+ci, 16lq+8a+kh, 8s+kw]   m=(ci,kh,kw,lq)
  V[m, d]   = v_pad[16h+ci, 16lq+8a+kh, 8s+kw]
  Q^T[d, n] = q[16h+d, n]
Softmax/PV are invariant to any fixed permutation of m, so we use our own
enumeration  m' = ((((khH*3+khL)*4+lq)*6+kw1)*2+kw0)*16+ci  (kh=3khH+khL,
kw=2kw1+kw0), which makes the K/V gather DMAs contiguous 192-element runs.

Sharding: 8 cores = 4 heads x 2 query-halves (n in [0,2048) / [2048,4096)).

Perf structure (v2):
  - LN stats via N=1 matmuls (s1/s2 columns), rstd = Exp(-0.5*Ln(var+eps))
    -> single ACT table set (natural_log_exp) for the whole kernel.
  - k/v images staged fp16; gathers lq-merged (24 DMAs) and spread across
    sync/vector/gpsimd queues; K^T replicated at partition bases 0/32/64/96.
  - Main loop ACT-bound: per group of 3 key-tiles, 3 row-packed S matmuls
    (tile_position=(32r,0), K=16 contraction) -> one exp [128,1536] -> 3 PV
    matmuls col-packed at (0,32*nck).  Emission is software-pipelined so the
    scalar engine's exp stream never waits on the PE.
  - V^T tile transposes for later chunks interleave into main-loop PE slack.
  - 1/den via Exp(-Ln(den)) on ACT (DVE reciprocal is slow; ACT rsqrt banned).
"""

import sys

for _p in ("/opt/trn_rl_repo", "/root/.axon_site/_ro/pypackages"):
    if _p not in sys.path:
        sys.path.insert(0, _p)

import numpy as np

import concourse.bass as bass
import concourse.mybir as mybir
import concourse.tile as tile
from concourse import bacc
from concourse.bass_utils import run_bass_kernel_spmd

F32 = mybir.dt.float32
F16 = mybir.dt.float16  # fp16: same PE rate as bf16, 8x mantissa
AF = mybir.ActivationFunctionType

HEADS, DH = 4, 16
NPIX, NHALF = 4096, 2048
PADW = 68          # padded image height/width
PFREE = PADW * 16  # pad free size: col*16 + ci = 1088
M = 9216           # keys per head
NT = 72            # m' tiles of 128
NCHUNK = 4         # khH chunks
TCH = NT // NCHUNK     # 18 key-tiles per chunk
MCH = M // NCHUNK      # 2304 m' per chunk
EPS = 1e-5
ROWPACK = True

_CACHE = {}


def _build(reps=1):
    nc = bacc.Bacc(trn_type="TRN2")

    x_d = nc.dram_tensor("x", [64, NPIX], F32, kind="ExternalInput")
    xq_d = nc.dram_tensor("xq", [64, NHALF], F32, kind="ExternalInput")
    wkvT_d = nc.dram_tensor("wkvT", [64, 32], F32, kind="ExternalInput")
    wqT_d = nc.dram_tensor("wqT", [64, 16], F32, kind="ExternalInput")
    woutT_d = nc.dram_tensor("woutT", [16, 64], F32, kind="ExternalInput")
    ones1_d = nc.dram_tensor("ones1", [1, 64], F32, kind="ExternalInput")
    ones64_d = nc.dram_tensor("ones64", [64, 1], F32, kind="ExternalInput")
    id128_d = nc.dram_tensor("id128", [128, 128], F32, kind="ExternalInput")
    id17_d = nc.dram_tensor("id17", [17, 17], F16, kind="ExternalInput")
    onesM_d = nc.dram_tensor("onesM", [1, M], F16, kind="ExternalInput")
    y_d = nc.dram_tensor("y", [64, NHALF], F32, kind="ExternalOutput")
    ktmp_d = nc.dram_tensor("ktmp", [NPIX, 16], F16)
    vtmp_d = nc.dram_tensor("vtmp", [NPIX, 16], F16)

    with tile.TileContext(nc) as tc:
        with tc.tile_pool(name="sb", bufs=1) as sb:
            # persistent sbuf tensors
            xsb = sb.tile([64, NPIX], F32)
            xqsb = sb.tile([64, NHALF], F32)
            xx = sb.tile([64, NPIX], F32)
            xxq = sb.tile([64, NHALF], F32)
            padk = sb.tile([PADW, PFREE], F16)
            padv = sb.tile([PADW, PFREE], F16)
            gkb4 = [sb.tile([128, MCH], F16, name=f"gkb4_{c}")
                    for c in range(NCHUNK)]
            gvc = [sb.tile([17, MCH], F16, name=f"gvc_{c}")
                   for c in range(NCHUNK)]
            vtc = [sb.tile([128, 17 * TCH], F16, name=f"vtc_{c}")
                   for c in range(NCHUNK)]
            qsb4 = sb.tile([128, NHALF], F16)
            stgkv = sb.tile([128, 1024], F32)
            stg16 = sb.tile([128, 1024], F16)
            rstdT = sb.tile([128, 48], F32)   # col t: chunk t (32 x, 16 xq)
            osb = sb.tile([128, 512], F32)
            den4 = sb.tile([4, 512], F32)
            dinv = sb.tile([4, 512], F32)
            num = sb.tile([16, 512], F32)
            den1 = sb.tile([1, 512], F32)
            bsb = sb.tile([64, 512], F32)
            ysb = sb.tile([64, NHALF], F32)
            wkvT = sb.tile([64, 32], F32)
            wqT = sb.tile([64, 16], F32)
            woutT = sb.tile([16, 64], F32)
            ones1 = sb.tile([1, 64], F32)
            ones64 = sb.tile([64, 1], F32)
            id128 = sb.tile([128, 128], F32)
            id17 = sb.tile([17, 17], F16)

            # input DMAs spread over queues
            nc.sync.dma_start(out=xsb[:, :], in_=x_d[:, :])
            nc.sync.dma_start(out=xqsb[:, :], in_=xq_d[:, :])
            for dst, src in ((wkvT, wkvT_d), (wqT, wqT_d), (woutT, woutT_d),
                             (ones1, ones1_d), (ones64, ones64_d)):
                nc.scalar.dma_start(out=dst[:, :], in_=src[:, :])
            nc.gpsimd.dma_start(out=id128[:, :], in_=id128_d[:, :])
            nc.gpsimd.dma_start(out=id17[:, :], in_=id17_d[:, :])
            nc.gpsimd.memset(padk[:, :], 0.0)
            nc.gpsimd.memset(padv[:, :], 0.0)
            for c in range(NCHUNK):
                nc.gpsimd.dma_start(out=gvc[c][16:17, :],
                                    in_=onesM_d[:, MCH * c:MCH * (c + 1)])

            with tc.tile_pool(name="sm", bufs=4) as sm, \
                 tc.tile_pool(name="pre", bufs=2, space="PSUM") as pre:

                # ---- LN stats: s1/s2 per 128-pixel chunk via N=1 matmuls ---
                nc.vector.tensor_mul(xx[:, :], xsb[:, :], xsb[:, :])
                nc.vector.tensor_mul(xxq[:, :], xqsb[:, :], xqsb[:, :])

                def chunk_src(t, sq):
                    a, b = (xx, xxq) if sq else (xsb, xqsb)
                    if t < 32:
                        return a[:, 128 * t:128 * (t + 1)]
                    return b[:, 128 * (t - 32):128 * (t - 31)]

                s12p = pre.tile([128, 96], F32, tag="st12")
                for t in range(48):
                    nc.tensor.matmul(s12p[:, t:t + 1], chunk_src(t, False),
                                     ones64[:, :], start=True, stop=True)
                for t in range(48):
                    nc.tensor.matmul(s12p[:, 48 + t:49 + t], chunk_src(t, True),
                                     ones64[:, :], start=True, stop=True)

                # rstd = exp(-0.5*ln(s2/64 - (s1/64)^2 + eps))
                mean = sm.tile([128, 48], F32, tag="mean")
                nc.vector.tensor_scalar_mul(mean[:, :], s12p[:, 0:48], 1.0 / 64)
                nc.vector.tensor_mul(mean[:, :], mean[:, :], mean[:, :])
                varr = sm.tile([128, 48], F32, tag="varr")
                nc.vector.tensor_scalar_mul(varr[:, :], s12p[:, 48:96], 1.0 / 64)
                nc.vector.tensor_sub(varr[:, :], varr[:, :], mean[:, :])
                nc.vector.tensor_scalar_add(varr[:, :], varr[:, :], EPS)
                lnv = sm.tile([128, 48], F32, tag="lnv")
                nc.scalar.activation(lnv[:, :], varr[:, :], AF.Ln)
                nc.scalar.activation(rstdT[:, :], lnv[:, :], AF.Exp, scale=-0.5)

                # ---- k,v 1x1 conv on RAW x; rstd folded in psum scaling ----
                for t in range(32):
                    kv = pre.tile([128, 32], F32, tag="kv")
                    nc.tensor.matmul(kv[:, :], xsb[:, 128 * t:128 * (t + 1)],
                                     wkvT[:, :], start=True, stop=True)
                    nc.vector.tensor_scalar_mul(
                        stgkv[:, 32 * t:32 * (t + 1)], kv[:, :],
                        rstdT[:, t:t + 1])
                nc.vector.tensor_copy(stg16[:, :], stgkv[:, :])

                # stg16[p, 32t + c0 + ci] = (k|v)[ci, pixel=128t+p]
                # -> (k|v)tmp[pixel, ci]  (DRAM, pixel-major, fp16)
                for tmp_d, c0 in ((ktmp_d, 0), (vtmp_d, 16)):
                    src_ap = bass.AP(tensor=stg16.tensor, offset=c0,
                                     ap=[[1024, 128], [32, 32], [1, 16]])
                    dst_ap = bass.AP(tensor=tmp_d, offset=0,
                                     ap=[[16, 128], [2048, 32], [1, 16]])
                    nc.sync.dma_start(out=dst_ap, in_=src_ap)
                # -> pad[row, (col+2)*16 + ci] interior (rows/cols +2 offset)
                for tmp_d, pad_t, eng in ((ktmp_d, padk, nc.sync),
                                          (vtmp_d, padv, nc.gpsimd)):
                    src_ap = bass.AP(tensor=tmp_d, offset=0,
                                     ap=[[1024, 64], [1, 1024]])
                    dst_ap = bass.AP(tensor=pad_t.tensor,
                                     offset=2 * PFREE + 2 * 16,
                                     ap=[[PFREE, 64], [1, 1024]])
                    eng.dma_start(out=dst_ap, in_=src_ap)

                # ---- q conv (head slice, 0.25 prefolded), pixel-part ------
                for t in range(16):
                    qp = pre.tile([128, 16], F32, tag="qp")
                    nc.tensor.matmul(qp[:, :], xqsb[:, 128 * t:128 * (t + 1)],
                                     wqT[:, :], start=True, stop=True)
                    qTc = sm.tile([128, 16], F32, tag="qTc")
                    nc.vector.tensor_scalar_mul(qTc[:, :], qp[:, :],
                                                rstdT[:, 32 + t:33 + t])
                    qp2 = pre.tile([16, 128], F32, tag="qp2")
                    nc.tensor.transpose(qp2[:, :], qTc[:, :], id128[:, :])
                    nc.vector.tensor_copy(qsb4[0:16, 128 * t:128 * (t + 1)],
                                          qp2[:, :])
                if ROWPACK:
                    for r in (1, 2, 3):
                        nc.sync.dma_start(
                            out=bass.AP(tensor=qsb4.tensor, offset=32 * r * NHALF,
                                        ap=[[NHALF, 16], [1, NHALF]]),
                            in_=bass.AP(tensor=qsb4.tensor, offset=0,
                                        ap=[[NHALF, 16], [1, NHALF]]))

                # ---- gathers: pad -> gkb4/gv chunks (lq merged, fp16) ------
                # G[8a+s, m'] = pad[16lq+3khH+khL+8a, (8s+2kw1+kw0)*16+ci]
                for c in range(NCHUNK):
                    for khL in range(3):
                        for lq in range(4):
                            for pad_t, gdst, eng in (
                                    (padk, gkb4[c], nc.sync),
                                    (padv, gvc[c], nc.gpsimd)):
                                src_ap = bass.AP(
                                    tensor=pad_t.tensor,
                                    offset=(16 * lq + 3 * c + khL) * PFREE,
                                    ap=[[8 * PFREE, 2], [128, 8], [1, 192]])
                                dst_ap = bass.AP(
                                    tensor=gdst.tensor,
                                    offset=768 * khL + 192 * lq,
                                    ap=[[MCH, 16], [1, 192]])
                                eng.dma_start(out=dst_ap, in_=src_ap)
                    if ROWPACK:
                        for r in (1, 2, 3):
                            nc.gpsimd.dma_start(
                                out=bass.AP(tensor=gkb4[c].tensor,
                                            offset=32 * r * MCH,
                                            ap=[[MCH, 16], [1, MCH]]),
                                in_=bass.AP(tensor=gkb4[c].tensor, offset=0,
                                            ap=[[MCH, 16], [1, MCH]]))

            _run_main(nc, tc, sb, gkb4, gvc, vtc, qsb4, osb, den4, dinv, num,
                      den1, bsb, ysb, woutT, ones1, id17, y_d, reps)

    nc.compile()
    return nc


def _run_main(nc, tc, sb, gkb4, gvc, vtc, qsb4, osb, den4, dinv, num, den1,
              bsb, ysb, woutT, ones1, id17, y_d, reps=1):
    NG = 24  # groups of 3 key-tiles
    with tc.tile_pool(name="stp", bufs=2, space="PSUM") as stp, \
         tc.tile_pool(name="pvp", bufs=1, space="PSUM") as pvp, \
         tc.tile_pool(name="pvt", bufs=1, space="PSUM") as pvtp, \
         tc.tile_pool(name="pp", bufs=3) as pp:
        pv = pvp.tile([128, 512], F32)

        def do_vt(c, i):
            vtp = pvtp.tile([128, 17], F16, tag="vt")
            nc.tensor.transpose(vtp[:, :], gvc[c][:, 128 * i:128 * (i + 1)],
                                id17[:, :])
            nc.vector.tensor_copy(vtc[c][:, 17 * i:17 * (i + 1)], vtp[:, :])

        for i in range(TCH):
            do_vt(0, i)
        vt_tasks = [(c, i) for c in range(1, NCHUNK) for i in range(TCH)]
        vt_tasks.reverse()  # pop() from chunk 1 upward

        def emit_S(g, nck):
            st = stp.tile([128, 1536], F32, tag="s")
            for j in range(3):
                T = 3 * g + j
                c, tcix = T // TCH, T % TCH
                r = T % 4 if ROWPACK else 0
                nc.tensor.matmul(
                    st[:, 512 * j:512 * (j + 1)],
                    gkb4[c][32 * r:32 * r + 16, 128 * tcix:128 * (tcix + 1)],
                    qsb4[32 * r:32 * r + 16, 512 * nck:512 * (nck + 1)],
                    start=True, stop=True,
                    tile_position=(32 * r, 0) if ROWPACK else None)
            return st

        for _rep in range(reps):
            pend = emit_S(0, 0)
            for g in range(NG):
                for nck in range(4):
                    st = pend
                    pt = pp.tile([128, 1536], F16, tag="p")
                    nc.scalar.activation(pt[:, :], st[:, :], AF.Exp)
                    ng, nn = (g, nck + 1) if nck < 3 else (g + 1, 0)
                    if ng < NG:
                        pend = emit_S(ng, nn)
                    # fill PE idle slot (waiting on exp) with V^T transposes
                    if g < 8:
                        for _ in range(2):
                            if vt_tasks:
                                do_vt(*vt_tasks.pop())
                    for j in range(3):
                        T = 3 * g + j
                        c, tcix = T // TCH, T % TCH
                        nc.tensor.matmul(
                            pv[32 * nck:32 * nck + 17, :],
                            vtc[c][:, 17 * tcix:17 * (tcix + 1)],
                            pt[:, 512 * j:512 * (j + 1)],
                            start=(g == 0 and j == 0),
                            stop=(g == NG - 1 and j == 2),
                            tile_position=(0, 32 * nck))

        # ---- normalize + final 1x1 conv -------------------------------
        nc.vector.tensor_copy(osb[:, :], pv[:, :])
        # den strips live at partitions 32*nck+16; 1/den = exp(-ln(den))
        nc.sync.dma_start(
            out=den4[:, :],
            in_=bass.AP(tensor=osb.tensor, offset=16 * 512,
                        ap=[[32 * 512, 4], [1, 512]]))
        nc.vector.reciprocal(dinv[:, :], den4[:, :])
        for nck in range(4):
            nc.sync.dma_start(
                out=num[:, :],
                in_=bass.AP(tensor=osb.tensor, offset=32 * nck * 512,
                            ap=[[512, 16], [1, 512]]))
            nc.sync.dma_start(
                out=den1[:, :],
                in_=bass.AP(tensor=dinv.tensor, offset=nck * 512,
                            ap=[[512, 1], [1, 512]]))
            yp = stp.tile([64, 512], F32, tag="s")
            nc.tensor.matmul(yp[:, :], woutT[:, :], num[:, :],
                             start=True, stop=True)
            bp = stp.tile([64, 512], F32, tag="s")
            nc.tensor.matmul(bp[:, :], ones1[:, :], den1[:, :],
                             start=True, stop=True)
            # DVE has one PSUM read port: stage bp in SBUF first
            nc.vector.tensor_copy(bsb[:, :], bp[:, :])
            nc.vector.tensor_mul(ysb[:, 512 * nck:512 * (nck + 1)],
                                 yp[:, :], bsb[:, :])
        nc.sync.dma_start(out=y_d[:, :], in_=ysb[:, :])


def _get_nc():
    if "nc" not in _CACHE:
        _CACHE["nc"] = _build()
    return _CACHE["nc"]


def kernel(x, w_qkv, w_out, ln_w, _want_trace=False, _tmpdir=None):
    x = np.asarray(x, np.float32)
    w_qkv = np.asarray(w_qkv, np.float32)
    w_out = np.asarray(w_out, np.float32)
    ln_w = np.asarray(ln_w, np.float32)

    x2d = np.ascontiguousarray(x.reshape(64, NPIX))
    ones1 = np.ones((1, 64), np.float32)
    ones64 = np.ones((64, 1), np.float32)
    id128 = np.eye(128, dtype=np.float32)
    id17 = np.eye(17, dtype=np.float16)
    onesM = np.ones((1, M), np.float16)

    in_maps = []
    for c in range(8):
        h, half = c % 4, c // 4
        wq = w_qkv[16 * h:16 * h + 16, :]
        wk = w_qkv[64 + 16 * h:64 + 16 * h + 16, :]
        wv = w_qkv[128 + 16 * h:128 + 16 * h + 16, :]
        lw = ln_w[None, :]
        in_maps.append({
            "x": x2d,
            "xq": np.ascontiguousarray(x2d[:, NHALF * half:NHALF * (half + 1)]),
            "wkvT": np.ascontiguousarray(
                (np.concatenate([wk, wv], 0) * lw).T.astype(np.float32)),
            "wqT": np.ascontiguousarray((0.25 * wq * lw).T.astype(np.float32)),
            "woutT": np.ascontiguousarray(
                w_out[:, 16 * h:16 * h + 16].T.astype(np.float32)),
            "ones1": ones1,
            "ones64": ones64,
            "id128": id128,
            "id17": id17,
            "onesM": onesM,
        })

    nc = _get_nc()
    res = run_bass_kernel_spmd(nc, in_maps, list(range(8)), trace=_want_trace,
                               tmpdir=_tmpdir)
    if _want_trace:
        _CACHE["last_result"] = res

    y = np.empty((64, NPIX), np.float32)
    for half in range(2):
        acc = np.zeros((64, NHALF), np.float32)
        for h in range(4):
            acc += res.results[4 * half + h]["y"]
        y[:, NHALF * half:NHALF * (half + 1)] = acc
    return y.reshape(1, 64, 64, 64)
